# revision 65
# baseline (speedup 1.0000x reference)
"""Causal self-attention (B=4, T=2048, C=1024, H=16) on 8 trn2 NeuronCores.

Sharding: 2 heads per core for QKV+attention (tensor-parallel over heads);
per-batch AllToAlls redistribute per-head attention outputs into per-core row
slices for a row-parallel output projection. QKV projection of batch b+1 and
the projection of batch b-1 are interleaved into the ACT-bound attention
stream of batch b so the PE fills its exp-wait gaps.

All matmul operands are bf16 (f32 PSUM accumulate): this halves HBM/DMA
traffic, removes the fp32r 4x penalty on sub-256 free dims, and doubles DVE
throughput on elementwise tiles. V is produced token-major directly from the
QKV matmul (stationary = x tile, moving = W_v), removing all PE transposes.
The causal mask is applied by multiplying the exp'd diagonal block with a
precomputed 0/1 lower-triangular bf16 mask; the attention v-bias is folded
into the projection bias on the host (b_proj + b_v @ W_proj). DMAs are merged
into few large strided transfers split across the SP and Pool queues so no
single sequencer serializes the issue path.
"""

import math
from contextlib import ExitStack

import numpy as np

NCORES = 8
B, T, C = 4, 2048, 1024
H = 16
D = C // H  # 64
HPC = H // NCORES  # heads per core = 2
BT = B * T  # 8192
ROWS_PER_CORE = BT // NCORES  # 1024
NKT = T // 128  # 16 k-tiles per batch
QROWS = T // NCORES  # 256

_compiled = None


def _build(no_collective=False):
    import concourse.tile as tile
    from concourse import bacc, mybir

    f32 = mybir.dt.float32
    bf16 = mybir.dt.bfloat16

    nc = bacc.Bacc()

    # ---- DRAM I/O (per-core views; same kernel on all 8 cores) ----
    xt_d = nc.dram_tensor("xt", [C, BT], bf16, kind="ExternalInput")
    wqkv_d = nc.dram_tensor("wqkv", [C, 3 * 128], bf16, kind="ExternalInput")
    bqk_d = nc.dram_tensor("bqk", [128, 2], f32, kind="ExternalInput")
    wp_d = nc.dram_tensor("wp", [C, C], bf16, kind="ExternalInput")
    bp_d = nc.dram_tensor("bp", [1, C], f32, kind="ExternalInput")
    out_d = nc.dram_tensor("out", [ROWS_PER_CORE, C], f32, kind="ExternalOutput")

    # internal DRAM for the four collectives (one per batch; shard = 256 rows)
    y_loc = [nc.dram_tensor(f"y_loc{q}", [NCORES, 128, QROWS], bf16) for q in range(B)]
    y_all = [nc.dram_tensor(f"y_all{q}", [NCORES, 128, QROWS], bf16) for q in range(B)]

    xt_r = xt_d[:, :].rearrange("(j p) t -> p j t", p=128)  # [128, 8, BT]
    wqkv_r = wqkv_d[:, :].rearrange("(j p) f -> p j f", p=128)  # [128, 8, 384]
    wp_r = wp_d[:, :].rearrange("(j p) f -> p j f", p=128)  # [128, 8, 1024]

    with tile.TileContext(nc) as tc, ExitStack() as ctx:
        qkv_pool = ctx.enter_context(tc.tile_pool(name="qkv_pool", bufs=2))
        wpool = ctx.enter_context(tc.tile_pool(name="wpool", bufs=1))
        xt_pool = ctx.enter_context(tc.tile_pool(name="xt_pool", bufs=3))
        pt_pool = ctx.enter_context(tc.tile_pool(name="pt", bufs=12))
        r_pool = ctx.enter_context(tc.tile_pool(name="rp", bufs=8))
        yt_pool = ctx.enter_context(tc.tile_pool(name="yt", bufs=6))
        ytr_pool = ctx.enter_context(tc.tile_pool(name="ytr", bufs=6))
        out_pool = ctx.enter_context(tc.tile_pool(name="op", bufs=4))
        ya_pool = ctx.enter_context(tc.tile_pool(name="ya", bufs=4))
        ps_big = ctx.enter_context(tc.tile_pool(name="ps_big", bufs=3, space="PSUM"))
        ps_ya = ctx.enter_context(tc.tile_pool(name="ps_ya", bufs=1, space="PSUM"))
        ps_yb = ctx.enter_context(tc.tile_pool(name="ps_yb", bufs=1, space="PSUM"))

        # ---- weights + constants ----
        wq_sb = wpool.tile([128, 8, 3 * 128], bf16)
        bqk_sb = wpool.tile([128, 2], f32)

        def emit_wq_load():
            # q+k weights first on SP (ahead of xt) so the first matmul can
            # start as early as possible; the v part follows
            nc.sync.dma_start(out=wq_sb[:, :, 0:256], in_=wqkv_r[:, :, 0:256])
            nc.sync.dma_start(out=bqk_sb, in_=bqk_d[:, :])

        def emit_wqv_load():
            nc.sync.dma_start(out=wq_sb[:, :, 256:384], in_=wqkv_r[:, :, 256:384])

        # 0/1 lower-triangular (incl diagonal) bf16 mask in [k=partition,
        # q=free] orientation: keep q >= k.
        trimask = wpool.tile([128, 2, 128], bf16, tag="trimask")
        wp_sb = wpool.tile([128, 8, C], bf16)
        bp_row = wpool.tile([128, C], f32, tag="bp_row")
        bias_bc = wpool.tile([128, C], f32, tag="bias_bc")

        def emit_consts():
            nc.gpsimd.memset(trimask[:, :, :], 1.0)
            for s in range(2):
                nc.gpsimd.affine_select(
                    out=trimask[:, s, :],
                    in_=trimask[:, s, :],
                    compare_op=mybir.AluOpType.is_ge,
                    fill=0.0,
                    base=0,
                    pattern=[[1, 128]],
                    channel_multiplier=-1,
                )

        def emit_wp_load():
            nc.sync.dma_start(out=wp_sb[:, :, :], in_=wp_r[:, :, :])
            nc.sync.dma_start(out=bp_row[0:1, :], in_=bp_d[:, :])
            nc.gpsimd.partition_broadcast(bias_bc[:, :], bp_row[0:1, :])

        def phase1(b):
            """QKV projection for batch b (generator; 12 yields). qT/kT are
            feature-major [128=(s,d), T]; V is token-major [128=kpos, kt,
            130] with all-ones columns at 0 (slot 0) and 65 (slot 1) so the
            AV matmul's psum row 0 is the softmax denominator."""
            qT = qkv_pool.tile([128, T], bf16, tag="qT", name=f"qT{b}")
            kT = qkv_pool.tile([128, T], bf16, tag="kT", name=f"kT{b}")
            V = qkv_pool.tile([128, NKT, 130], bf16, tag="V", name=f"V{b}")
            nc.gpsimd.memset(V[:, :, 64], 1.0)
            nc.gpsimd.memset(V[:, :, 129], 1.0)
            result[b] = (qT, kT, V)

            xt_tiles = {}

            def load_xt(tt):
                tok0 = b * T + tt * 512
                xt_t = xt_pool.tile([128, 8, 512], bf16, tag="xt", name=f"xt{b}_{tt}")
                if b == 0 and tt == 0:
                    # split the cold-start load so the first j-chunks land early
                    nc.sync.dma_start(out=xt_t[:, 0:2, :], in_=xt_r[:, 0:2, tok0 : tok0 + 512])
                    nc.sync.dma_start(out=xt_t[:, 2:8, :], in_=xt_r[:, 2:8, tok0 : tok0 + 512])
                else:
                    nc.sync.dma_start(out=xt_t[:, :, :], in_=xt_r[:, :, tok0 : tok0 + 512])
                xt_tiles[tt] = xt_t

            if b == 0:
                emit_wq_load()
            load_xt(0)
            if b == 0:
                emit_wqv_load()
            for tt in range(4):  # 512-token tiles
                if tt + 1 < 4:
                    load_xt(tt + 1)  # prefetch one tile ahead
                xt_t = xt_tiles.pop(tt)
                for m in range(2):  # q, k feature chunks (feature-major)
                    ps = ps_big.tile([128, 2, 512], f32, tag="big")
                    for j in range(8):
                        nc.tensor.matmul(
                            ps[:, 0, :],
                            wq_sb[:, j, m * 128 : (m + 1) * 128],
                            xt_t[:, j, :],
                            start=(j == 0),
                            stop=(j == 7),
                        )
                        if j == 3:
                            yield
                    dst = qT if m == 0 else kT
                    nc.vector.tensor_scalar_add(
                        dst[:, tt * 512 : (tt + 1) * 512], ps[:, 0, :], bqk_sb[:, m : m + 1]
                    )
                    yield
                # v chunk, token-major: stationary = x tile, moving = W_v
                ps = ps_big.tile([128, 2, 512], f32, tag="big")
                for i in range(4):
                    for j in range(8):
                        nc.tensor.matmul(
                            ps[:, 0, i * 128 : (i + 1) * 128],
                            xt_t[:, j, i * 128 : (i + 1) * 128],
                            wq_sb[:, j, 256:384],
                            start=(j == 0),
                            stop=(j == 7),
                        )
                    kt_idx = tt * 4 + i
                    nc.vector.tensor_copy(
                        V[:, kt_idx, 0:64], ps[:, 0, i * 128 : i * 128 + 64]
                    )
                    nc.vector.tensor_copy(
                        V[:, kt_idx, 65:129], ps[:, 0, i * 128 + 64 : i * 128 + 128]
                    )
                    if i % 2 == 1:
                        yield

        def phase2(b):
            """Causal attention for batch b, both head-slots packed into each
            512-query supertile strip: one exp instruction covers both
            slots' scores via a 3D access pattern."""
            qT, kT, V = result[b]
            for qsup in range(4):  # 512-wide query supertiles
                yield True  # filler hint before the supertile's first strip
                q0 = qsup * 512
                ps_y = [
                    ps_ya.tile([128, 512], f32, tag="ya", name="ps_ya"),
                    ps_yb.tile([128, 512], f32, tag="yb", name="ps_yb"),
                ]
                nkt = 4 * (qsup + 1)
                pending = []

                def flush_one():
                    pt, kt, off = pending.pop(0)
                    for s in range(2):
                        nc.tensor.matmul(
                            ps_y[s][0:65, off:512],
                            V[:, kt, s * 65 : s * 65 + 65],
                            pt[:, s, off:512],
                            start=(kt == 0),
                            stop=(kt == nkt - 1),
                        )

                for kt in range(nkt):
                    off = max(0, kt * 128 - q0)
                    ps_s = ps_big.tile([128, 2, 512], f32, tag="big")
                    for s in range(2):
                        p0 = s * D
                        nc.tensor.matmul(
                            ps_s[:, s, off:512],
                            kT[p0 : p0 + D, kt * 128 : (kt + 1) * 128],
                            qT[p0 : p0 + D, q0 + off : q0 + 512],
                            start=True,
                            stop=True,
                        )
                    pt = pt_pool.tile([128, 2, 512], bf16)
                    nc.scalar.activation(
                        pt[:, :, off:512],
                        ps_s[:, :, off:512],
                        mybir.ActivationFunctionType.Exp,
                        scale=1.0 / math.sqrt(D),
                    )
                    if kt * 128 >= q0:  # diagonal strip -> causal mask,
                        # both slots in one op via the duplicated mask
                        nc.vector.tensor_mul(
                            pt[:, :, off : off + 128],
                            pt[:, :, off : off + 128],
                            trimask[:, :, :],
                        )
                    pending.append((pt, kt, off))
                    if len(pending) > 2:  # 2-strip AV skew
                        flush_one()
                    yield True
                while pending:
                    flush_one()

                # free the psum accumulators quickly with copies (row 64 is
                # the denominator), then normalize from SBUF off the
                # slot-critical path
                for s in range(2):
                    p0 = s * D
                    yt_raw = ytr_pool.tile([128, 512], bf16)
                    nc.vector.tensor_copy(yt_raw[0:65, :], ps_y[s][0:65, :])
                    r_t = r_pool.tile([128, 512], bf16, tag="r")
                    with nc.allow_low_precision(reason="softmax denom recip bf16"):
                        nc.vector.reciprocal(r_t[0:1, :], yt_raw[64:65, :])
                    rb_t = r_pool.tile([128, 512], bf16, tag="rb")
                    nc.gpsimd.partition_broadcast(rb_t[0:64, :], r_t[0:1, :])
                    yt_sb = yt_pool.tile([128, 512], bf16)
                    nc.vector.tensor_mul(yt_sb[0:64, :], yt_raw[0:64, :], rb_t[0:64, :])
                    nc.sync.dma_start(
                        out=y_loc[b][qsup * 2 : (qsup + 1) * 2, p0 : p0 + D, :].rearrange(
                            "s p c -> p s c"
                        ),
                        in_=yt_sb[0:64, :],
                    )

        def emit_a2a(q):
            if no_collective:
                return
            nc.gpsimd.collective_compute(
                "AllToAll",
                mybir.AluOpType.bypass,
                replica_groups=[list(range(NCORES))],
                ins=[y_loc[q][:, :, :]],
                outs=[y_all[q][:, :, :]],
            )

        ya_tiles = {}
        warm_src = [None]

        def emit_ya_load(q):
            """Issue the y redistribution load right after batch q's A2A so
            proj never head-of-line-blocks the PE queue on it."""
            y_src = y_loc[q] if no_collective else y_all[q]
            ya = ya_pool.tile([128, 8, QROWS], bf16, tag="ya", name=f"ya{q}")
            nc.sync.dma_start(out=ya, in_=y_src[:, :, :].rearrange("i p t -> p i t"))
            ya_tiles[q] = ya

        def proj(q):
            """Output projection for this core's 256 rows of batch q.
            Generator: yields after each of 2 row-tiles."""
            ya = ya_tiles.pop(q)
            warm_src[0] = ya
            for rt in range(2):
                ps_o = ps_big.tile([128, 2, 512], f32, tag="big")
                out_sb = out_pool.tile([128, 1024], f32)
                row = q * 256 + rt * 128
                # seg-outer so each 512-half drains while the next computes
                for g, (lo, hi) in enumerate(((0, 512), (512, 1024))):
                    for i in range(8):  # feature chunks (source cores)
                        nc.tensor.matmul(
                            ps_o[:, g, :],
                            ya[:, i, rt * 128 : (rt + 1) * 128],
                            wp_sb[:, i, lo:hi],
                            start=(i == 0),
                            stop=(i == 7),
                        )
                    nc.vector.tensor_add(
                        out_sb[:, lo:hi], ps_o[:, g, :], bias_bc[:, lo:hi]
                    )
                    nc.sync.dma_start(
                        out=out_d[row : row + 128, lo:hi], in_=out_sb[:, lo:hi]
                    )
                yield

        def run_interleaved(primary, filler, pace=(1, 1, 0)):
            """Drain `primary`, advancing `filler` at hinted insertion
            points, pacing feeds by the cyclic `pace` pattern so filler
            work spreads across the whole attention stream instead of
            front-loading and running dry."""
            i = 0
            for hint in primary:
                if filler is not None and hint and pace[i % len(pace)]:
                    try:
                        next(filler)
                    except StopIteration:
                        filler = None
                i += 1
            return filler

        def drain(gen):
            if gen is not None:
                for _ in gen:
                    pass

        def chain(*gens):
            for g in gens:
                if g is not None:
                    yield from g

        def take(gen, n):
            for _ in range(n):
                try:
                    next(gen)
                except StopIteration:
                    return
                yield

        result = {}
        # startup: only the first half (tt0, tt1) of batch 0's qkv stands
        # alone. Every batch's attention then gets as filler: the second
        # half of its OWN qkv (tt2, tt3 — the batch-0 software-pipeline
        # pattern applied to all batches), the first half of the NEXT
        # batch's qkv, and the projection of batch b-1. This spreads PE
        # filler evenly so the ACT-bound attention never starves the PE —
        # including batch 3, which previously ran dry.
        p1 = {0: phase1(0)}
        next(p1[0])
        emit_consts()
        for _ in range(11):
            next(p1[0])
        for b in range(B):
            parts = [p1[b]]  # rest of this batch's qkv (tt2, tt3)
            if b < B - 1:
                p1[b + 1] = phase1(b + 1)
                parts.append(take(p1[b + 1], 16))  # next batch's tt0, tt1
            if 1 <= b <= 2:
                parts.append(proj(b - 1))
            filler = chain(*parts)
            pace = (1, 0) if b < 3 else (1, 0, 0)
            filler = run_interleaved(phase2(b), filler, pace)
            drain(filler)
            if b == 0:
                emit_wp_load()  # off the critical startup path
            emit_a2a(b)
            emit_ya_load(b)
        drain(proj(2))
        ya2 = warm_src[0]
        for w in range(1):
            warm = ps_big.tile([128, 1024], f32, tag="big")
            for i in range(8):
                nc.tensor.matmul(
                    warm[:, 0:512],
                    ya2[:, i, 0:128],
                    wp_sb[:, i, 0:512],
                    start=(i == 0),
                    stop=(i == 7),
                )
                nc.tensor.matmul(
                    warm[:, 512:1024],
                    ya2[:, i, 0:128],
                    wp_sb[:, i, 512:1024],
                    start=(i == 0),
                    stop=(i == 7),
                )
        drain(proj(3))

    nc.compile()
    return nc


def _get_compiled():
    global _compiled
    if _compiled is None:
        _compiled = _build()
    return _compiled


def _bf16(a):
    import ml_dtypes

    return np.ascontiguousarray(a).astype(ml_dtypes.bfloat16)


def _make_in_maps(x, W_attn, b_attn, W_proj, b_proj):
    xt = _bf16(x.reshape(BT, C).T)  # [C, BT] bf16, shared across cores
    wp = _bf16(W_proj)
    # fold the attention v-bias into the projection bias:
    # y_final = (attn_out + b_v) @ W_proj + b_proj
    bp = np.ascontiguousarray(
        (b_proj + b_attn[2 * C : 3 * C] @ W_proj).reshape(1, C).astype(np.float32)
    )
    in_maps = []
    for c in range(NCORES):
        heads = [HPC * c + s for s in range(HPC)]
        cols = []
        for m in range(3):  # q, k, v blocks of W_attn
            for h in heads:
                cols.extend(range(m * C + h * D, m * C + (h + 1) * D))
        cols = np.asarray(cols)
        bqk = np.ascontiguousarray(
            b_attn[cols].reshape(3, 128).T[:, 0:2].astype(np.float32)
        )
        in_maps.append(
            {
                "xt": xt,
                "wqkv": _bf16(W_attn[:, cols]),
                "bqk": bqk,
                "wp": wp,
                "bp": bp,
            }
        )
    return in_maps


def kernel(x, W_attn, b_attn, W_proj, b_proj):
    from concourse.bass_utils import run_bass_kernel_spmd

    x = np.asarray(x, dtype=np.float32)
    W_attn = np.asarray(W_attn, dtype=np.float32)
    b_attn = np.asarray(b_attn, dtype=np.float32)
    W_proj = np.asarray(W_proj, dtype=np.float32)
    b_proj = np.asarray(b_proj, dtype=np.float32)

    nc = _get_compiled()
    in_maps = _make_in_maps(x, W_attn, b_attn, W_proj, b_proj)
    res = run_bass_kernel_spmd(nc, in_maps, core_ids=list(range(NCORES)))

    # core c's output: for each batch q, rows [256c, 256c+256) of that batch
    out = np.empty((BT, C), dtype=np.float32)
    for c in range(NCORES):
        o = res.results[c]["out"]
        for q in range(B):
            out[2048 * q + 256 * c : 2048 * q + 256 * (c + 1)] = o[256 * q : 256 * (q + 1)]
    return out.reshape(B, T, C)


# revision 66
# speedup vs baseline: 1.0000x; 1.0000x over previous
"""Causal self-attention (B=4, T=2048, C=1024, H=16) on 8 trn2 NeuronCores.

Sharding: 2 heads per core for QKV+attention (tensor-parallel over heads);
per-batch AllToAlls redistribute per-head attention outputs into per-core row
slices for a row-parallel output projection. QKV projection of batch b+1 and
the projection of batch b-1 are interleaved into the ACT-bound attention
stream of batch b so the PE fills its exp-wait gaps.

All matmul operands are bf16 (f32 PSUM accumulate): this halves HBM/DMA
traffic, removes the fp32r 4x penalty on sub-256 free dims, and doubles DVE
throughput on elementwise tiles. V is produced token-major directly from the
QKV matmul (stationary = x tile, moving = W_v), removing all PE transposes.
The causal mask is applied by multiplying the exp'd diagonal block with a
precomputed 0/1 lower-triangular bf16 mask; the attention v-bias is folded
into the projection bias on the host (b_proj + b_v @ W_proj). DMAs are merged
into few large strided transfers split across the SP and Pool queues so no
single sequencer serializes the issue path.
"""

import math
from contextlib import ExitStack

import numpy as np

NCORES = 8
B, T, C = 4, 2048, 1024
H = 16
D = C // H  # 64
HPC = H // NCORES  # heads per core = 2
BT = B * T  # 8192
ROWS_PER_CORE = BT // NCORES  # 1024
NKT = T // 128  # 16 k-tiles per batch
QROWS = T // NCORES  # 256

_compiled = None


def _build(no_collective=False):
    import concourse.tile as tile
    from concourse import bacc, mybir

    f32 = mybir.dt.float32
    bf16 = mybir.dt.bfloat16

    nc = bacc.Bacc()

    # ---- DRAM I/O (per-core views; same kernel on all 8 cores) ----
    xt_d = nc.dram_tensor("xt", [C, BT], bf16, kind="ExternalInput")
    wqkv_d = nc.dram_tensor("wqkv", [C, 3 * 128], bf16, kind="ExternalInput")
    bqk_d = nc.dram_tensor("bqk", [128, 2], f32, kind="ExternalInput")
    wp_d = nc.dram_tensor("wp", [C, C], bf16, kind="ExternalInput")
    bp_d = nc.dram_tensor("bp", [1, C], f32, kind="ExternalInput")
    out_d = nc.dram_tensor("out", [ROWS_PER_CORE, C], f32, kind="ExternalOutput")

    # internal DRAM for the four collectives (one per batch; shard = 256 rows)
    y_loc = [nc.dram_tensor(f"y_loc{q}", [NCORES, 128, QROWS], bf16) for q in range(B)]
    y_all = [nc.dram_tensor(f"y_all{q}", [NCORES, 128, QROWS], bf16) for q in range(B)]

    xt_r = xt_d[:, :].rearrange("(j p) t -> p j t", p=128)  # [128, 8, BT]
    wqkv_r = wqkv_d[:, :].rearrange("(j p) f -> p j f", p=128)  # [128, 8, 384]
    wp_r = wp_d[:, :].rearrange("(j p) f -> p j f", p=128)  # [128, 8, 1024]

    with tile.TileContext(nc) as tc, ExitStack() as ctx:
        qkv_pool = ctx.enter_context(tc.tile_pool(name="qkv_pool", bufs=2))
        wpool = ctx.enter_context(tc.tile_pool(name="wpool", bufs=1))
        xt_pool = ctx.enter_context(tc.tile_pool(name="xt_pool", bufs=3))
        pt_pool = ctx.enter_context(tc.tile_pool(name="pt", bufs=12))
        r_pool = ctx.enter_context(tc.tile_pool(name="rp", bufs=8))
        yt_pool = ctx.enter_context(tc.tile_pool(name="yt", bufs=6))
        ytr_pool = ctx.enter_context(tc.tile_pool(name="ytr", bufs=6))
        out_pool = ctx.enter_context(tc.tile_pool(name="op", bufs=4))
        ya_pool = ctx.enter_context(tc.tile_pool(name="ya", bufs=4))
        ps_big = ctx.enter_context(tc.tile_pool(name="ps_big", bufs=3, space="PSUM"))
        ps_ya = ctx.enter_context(tc.tile_pool(name="ps_ya", bufs=1, space="PSUM"))
        ps_yb = ctx.enter_context(tc.tile_pool(name="ps_yb", bufs=1, space="PSUM"))

        # ---- weights + constants ----
        wq_sb = wpool.tile([128, 8, 3 * 128], bf16)
        bqk_sb = wpool.tile([128, 2], f32)

        def emit_wq_load():
            # q+k weights first on SP (ahead of xt) so the first matmul can
            # start as early as possible; the v part follows
            nc.sync.dma_start(out=wq_sb[:, :, 0:256], in_=wqkv_r[:, :, 0:256])
            nc.sync.dma_start(out=bqk_sb, in_=bqk_d[:, :])

        def emit_wqv_load():
            nc.sync.dma_start(out=wq_sb[:, :, 256:384], in_=wqkv_r[:, :, 256:384])

        # 0/1 lower-triangular (incl diagonal) bf16 mask in [k=partition,
        # q=free] orientation: keep q >= k.
        trimask = wpool.tile([128, 2, 128], bf16, tag="trimask")
        wp_sb = wpool.tile([128, 8, C], bf16)
        bp_row = wpool.tile([128, C], f32, tag="bp_row")
        bias_bc = wpool.tile([128, C], f32, tag="bias_bc")

        def emit_consts():
            nc.gpsimd.memset(trimask[:, :, :], 1.0)
            for s in range(2):
                nc.gpsimd.affine_select(
                    out=trimask[:, s, :],
                    in_=trimask[:, s, :],
                    compare_op=mybir.AluOpType.is_ge,
                    fill=0.0,
                    base=0,
                    pattern=[[1, 128]],
                    channel_multiplier=-1,
                )

        def emit_wp_load():
            nc.sync.dma_start(out=wp_sb[:, :, :], in_=wp_r[:, :, :])
            nc.sync.dma_start(out=bp_row[0:1, :], in_=bp_d[:, :])
            nc.gpsimd.partition_broadcast(bias_bc[:, :], bp_row[0:1, :])

        def phase1(b):
            """QKV projection for batch b (generator; 12 yields). qT/kT are
            feature-major [128=(s,d), T]; V is token-major [128=kpos, kt,
            130] with all-ones columns at 0 (slot 0) and 65 (slot 1) so the
            AV matmul's psum row 0 is the softmax denominator."""
            qT = qkv_pool.tile([128, T], bf16, tag="qT", name=f"qT{b}")
            kT = qkv_pool.tile([128, T], bf16, tag="kT", name=f"kT{b}")
            V = qkv_pool.tile([128, NKT, 130], bf16, tag="V", name=f"V{b}")
            nc.gpsimd.memset(V[:, :, 64], 1.0)
            nc.gpsimd.memset(V[:, :, 129], 1.0)
            result[b] = (qT, kT, V)

            xt_tiles = {}

            def load_xt(tt):
                tok0 = b * T + tt * 512
                xt_t = xt_pool.tile([128, 8, 512], bf16, tag="xt", name=f"xt{b}_{tt}")
                if b == 0 and tt == 0:
                    # split the cold-start load so the first j-chunks land early
                    nc.sync.dma_start(out=xt_t[:, 0:2, :], in_=xt_r[:, 0:2, tok0 : tok0 + 512])
                    nc.sync.dma_start(out=xt_t[:, 2:8, :], in_=xt_r[:, 2:8, tok0 : tok0 + 512])
                else:
                    nc.sync.dma_start(out=xt_t[:, :, :], in_=xt_r[:, :, tok0 : tok0 + 512])
                xt_tiles[tt] = xt_t

            if b == 0:
                emit_wq_load()
            load_xt(0)
            if b == 0:
                emit_wqv_load()
            for tt in range(4):  # 512-token tiles
                if tt + 1 < 4:
                    load_xt(tt + 1)  # prefetch one tile ahead
                xt_t = xt_tiles.pop(tt)
                for m in range(2):  # q, k feature chunks (feature-major)
                    ps = ps_big.tile([128, 2, 512], f32, tag="big")
                    for j in range(8):
                        nc.tensor.matmul(
                            ps[:, 0, :],
                            wq_sb[:, j, m * 128 : (m + 1) * 128],
                            xt_t[:, j, :],
                            start=(j == 0),
                            stop=(j == 7),
                        )
                        if j == 3:
                            yield
                    dst = qT if m == 0 else kT
                    nc.vector.tensor_scalar_add(
                        dst[:, tt * 512 : (tt + 1) * 512], ps[:, 0, :], bqk_sb[:, m : m + 1]
                    )
                    yield
                # v chunk, token-major: stationary = x tile, moving = W_v
                ps = ps_big.tile([128, 2, 512], f32, tag="big")
                for i in range(4):
                    for j in range(8):
                        nc.tensor.matmul(
                            ps[:, 0, i * 128 : (i + 1) * 128],
                            xt_t[:, j, i * 128 : (i + 1) * 128],
                            wq_sb[:, j, 256:384],
                            start=(j == 0),
                            stop=(j == 7),
                        )
                    kt_idx = tt * 4 + i
                    nc.vector.tensor_copy(
                        V[:, kt_idx, 0:64], ps[:, 0, i * 128 : i * 128 + 64]
                    )
                    nc.vector.tensor_copy(
                        V[:, kt_idx, 65:129], ps[:, 0, i * 128 + 64 : i * 128 + 128]
                    )
                    if i % 2 == 1:
                        yield

        def phase2(b):
            """Causal attention for batch b, both head-slots packed into each
            512-query supertile strip: one exp instruction covers both
            slots' scores via a 3D access pattern."""
            qT, kT, V = result[b]
            for qsup in range(4):  # 512-wide query supertiles
                yield True  # filler hint before the supertile's first strip
                q0 = qsup * 512
                ps_y = [
                    ps_ya.tile([128, 512], f32, tag="ya", name="ps_ya"),
                    ps_yb.tile([128, 512], f32, tag="yb", name="ps_yb"),
                ]
                nkt = 4 * (qsup + 1)
                pending = []

                def flush_one():
                    pt, kt, off = pending.pop(0)
                    for s in range(2):
                        nc.tensor.matmul(
                            ps_y[s][0:65, off:512],
                            V[:, kt, s * 65 : s * 65 + 65],
                            pt[:, s, off:512],
                            start=(kt == 0),
                            stop=(kt == nkt - 1),
                        )

                for kt in range(nkt):
                    off = max(0, kt * 128 - q0)
                    ps_s = ps_big.tile([128, 2, 512], f32, tag="big")
                    for s in range(2):
                        p0 = s * D
                        nc.tensor.matmul(
                            ps_s[:, s, off:512],
                            kT[p0 : p0 + D, kt * 128 : (kt + 1) * 128],
                            qT[p0 : p0 + D, q0 + off : q0 + 512],
                            start=True,
                            stop=True,
                        )
                    pt = pt_pool.tile([128, 2, 512], bf16)
                    nc.scalar.activation(
                        pt[:, :, off:512],
                        ps_s[:, :, off:512],
                        mybir.ActivationFunctionType.Exp,
                        scale=1.0 / math.sqrt(D),
                    )
                    if kt * 128 >= q0:  # diagonal strip -> causal mask,
                        # both slots in one op via the duplicated mask
                        nc.vector.tensor_mul(
                            pt[:, :, off : off + 128],
                            pt[:, :, off : off + 128],
                            trimask[:, :, :],
                        )
                    pending.append((pt, kt, off))
                    if len(pending) > 3:  # 3-strip AV skew
                        flush_one()
                    yield True
                while pending:
                    flush_one()

                # free the psum accumulators quickly with copies (row 64 is
                # the denominator), then normalize from SBUF off the
                # slot-critical path
                for s in range(2):
                    p0 = s * D
                    yt_raw = ytr_pool.tile([128, 512], bf16)
                    nc.vector.tensor_copy(yt_raw[0:65, :], ps_y[s][0:65, :])
                    r_t = r_pool.tile([128, 512], bf16, tag="r")
                    with nc.allow_low_precision(reason="softmax denom recip bf16"):
                        nc.vector.reciprocal(r_t[0:1, :], yt_raw[64:65, :])
                    rb_t = r_pool.tile([128, 512], bf16, tag="rb")
                    nc.gpsimd.partition_broadcast(rb_t[0:64, :], r_t[0:1, :])
                    yt_sb = yt_pool.tile([128, 512], bf16)
                    nc.vector.tensor_mul(yt_sb[0:64, :], yt_raw[0:64, :], rb_t[0:64, :])
                    nc.sync.dma_start(
                        out=y_loc[b][qsup * 2 : (qsup + 1) * 2, p0 : p0 + D, :].rearrange(
                            "s p c -> p s c"
                        ),
                        in_=yt_sb[0:64, :],
                    )

        def emit_a2a(q):
            if no_collective:
                return
            nc.gpsimd.collective_compute(
                "AllToAll",
                mybir.AluOpType.bypass,
                replica_groups=[list(range(NCORES))],
                ins=[y_loc[q][:, :, :]],
                outs=[y_all[q][:, :, :]],
            )

        ya_tiles = {}
        warm_src = [None]

        def emit_ya_load(q):
            """Issue the y redistribution load right after batch q's A2A so
            proj never head-of-line-blocks the PE queue on it."""
            y_src = y_loc[q] if no_collective else y_all[q]
            ya = ya_pool.tile([128, 8, QROWS], bf16, tag="ya", name=f"ya{q}")
            nc.sync.dma_start(out=ya, in_=y_src[:, :, :].rearrange("i p t -> p i t"))
            ya_tiles[q] = ya

        def proj(q):
            """Output projection for this core's 256 rows of batch q.
            Generator: yields after each of 2 row-tiles."""
            ya = ya_tiles.pop(q)
            warm_src[0] = ya
            for rt in range(2):
                ps_o = ps_big.tile([128, 2, 512], f32, tag="big")
                out_sb = out_pool.tile([128, 1024], f32)
                row = q * 256 + rt * 128
                # seg-outer so each 512-half drains while the next computes
                for g, (lo, hi) in enumerate(((0, 512), (512, 1024))):
                    for i in range(8):  # feature chunks (source cores)
                        nc.tensor.matmul(
                            ps_o[:, g, :],
                            ya[:, i, rt * 128 : (rt + 1) * 128],
                            wp_sb[:, i, lo:hi],
                            start=(i == 0),
                            stop=(i == 7),
                        )
                    nc.vector.tensor_add(
                        out_sb[:, lo:hi], ps_o[:, g, :], bias_bc[:, lo:hi]
                    )
                    nc.sync.dma_start(
                        out=out_d[row : row + 128, lo:hi], in_=out_sb[:, lo:hi]
                    )
                yield

        def run_interleaved(primary, filler, pace=(1, 1, 0)):
            """Drain `primary`, advancing `filler` at hinted insertion
            points, pacing feeds by the cyclic `pace` pattern so filler
            work spreads across the whole attention stream instead of
            front-loading and running dry."""
            i = 0
            for hint in primary:
                if filler is not None and hint and pace[i % len(pace)]:
                    try:
                        next(filler)
                    except StopIteration:
                        filler = None
                i += 1
            return filler

        def drain(gen):
            if gen is not None:
                for _ in gen:
                    pass

        def chain(*gens):
            for g in gens:
                if g is not None:
                    yield from g

        def take(gen, n):
            for _ in range(n):
                try:
                    next(gen)
                except StopIteration:
                    return
                yield

        result = {}
        # startup: only the first half (tt0, tt1) of batch 0's qkv stands
        # alone. Every batch's attention then gets as filler: the second
        # half of its OWN qkv (tt2, tt3 — the batch-0 software-pipeline
        # pattern applied to all batches), the first half of the NEXT
        # batch's qkv, and the projection of batch b-1. This spreads PE
        # filler evenly so the ACT-bound attention never starves the PE —
        # including batch 3, which previously ran dry.
        p1 = {0: phase1(0)}
        next(p1[0])
        emit_consts()
        for _ in range(11):
            next(p1[0])
        for b in range(B):
            parts = [p1[b]]  # rest of this batch's qkv (tt2, tt3)
            if b < B - 1:
                p1[b + 1] = phase1(b + 1)
                parts.append(take(p1[b + 1], 12))  # next batch's tt0, tt1
            if 1 <= b <= 2:
                parts.append(proj(b - 1))
            filler = chain(*parts)
            pace = (1, 0) if b < 3 else (1, 0, 0)
            filler = run_interleaved(phase2(b), filler, pace)
            drain(filler)
            if b == 0:
                emit_wp_load()  # off the critical startup path
            emit_a2a(b)
            emit_ya_load(b)
        drain(proj(2))
        ya2 = warm_src[0]
        for w in range(1):
            warm = ps_big.tile([128, 1024], f32, tag="big")
            for i in range(8):
                nc.tensor.matmul(
                    warm[:, 0:512],
                    ya2[:, i, 0:128],
                    wp_sb[:, i, 0:512],
                    start=(i == 0),
                    stop=(i == 7),
                )
                nc.tensor.matmul(
                    warm[:, 512:1024],
                    ya2[:, i, 0:128],
                    wp_sb[:, i, 512:1024],
                    start=(i == 0),
                    stop=(i == 7),
                )
        drain(proj(3))

    nc.compile()
    return nc


def _get_compiled():
    global _compiled
    if _compiled is None:
        _compiled = _build()
    return _compiled


def _bf16(a):
    import ml_dtypes

    return np.ascontiguousarray(a).astype(ml_dtypes.bfloat16)


def _make_in_maps(x, W_attn, b_attn, W_proj, b_proj):
    xt = _bf16(x.reshape(BT, C).T)  # [C, BT] bf16, shared across cores
    wp = _bf16(W_proj)
    # fold the attention v-bias into the projection bias:
    # y_final = (attn_out + b_v) @ W_proj + b_proj
    bp = np.ascontiguousarray(
        (b_proj + b_attn[2 * C : 3 * C] @ W_proj).reshape(1, C).astype(np.float32)
    )
    in_maps = []
    for c in range(NCORES):
        heads = [HPC * c + s for s in range(HPC)]
        cols = []
        for m in range(3):  # q, k, v blocks of W_attn
            for h in heads:
                cols.extend(range(m * C + h * D, m * C + (h + 1) * D))
        cols = np.asarray(cols)
        bqk = np.ascontiguousarray(
            b_attn[cols].reshape(3, 128).T[:, 0:2].astype(np.float32)
        )
        in_maps.append(
            {
                "xt": xt,
                "wqkv": _bf16(W_attn[:, cols]),
                "bqk": bqk,
                "wp": wp,
                "bp": bp,
            }
        )
    return in_maps


def kernel(x, W_attn, b_attn, W_proj, b_proj):
    from concourse.bass_utils import run_bass_kernel_spmd

    x = np.asarray(x, dtype=np.float32)
    W_attn = np.asarray(W_attn, dtype=np.float32)
    b_attn = np.asarray(b_attn, dtype=np.float32)
    W_proj = np.asarray(W_proj, dtype=np.float32)
    b_proj = np.asarray(b_proj, dtype=np.float32)

    nc = _get_compiled()
    in_maps = _make_in_maps(x, W_attn, b_attn, W_proj, b_proj)
    res = run_bass_kernel_spmd(nc, in_maps, core_ids=list(range(NCORES)))

    # core c's output: for each batch q, rows [256c, 256c+256) of that batch
    out = np.empty((BT, C), dtype=np.float32)
    for c in range(NCORES):
        o = res.results[c]["out"]
        for q in range(B):
            out[2048 * q + 256 * c : 2048 * q + 256 * (c + 1)] = o[256 * q : 256 * (q + 1)]
    return out.reshape(B, T, C)


# revision 67
# speedup vs baseline: 1.0006x; 1.0006x over previous
"""Causal self-attention (B=4, T=2048, C=1024, H=16) on 8 trn2 NeuronCores.

Sharding: 2 heads per core for QKV+attention (tensor-parallel over heads);
per-batch AllToAlls redistribute per-head attention outputs into per-core row
slices for a row-parallel output projection. QKV projection of batch b+1 and
the projection of batch b-1 are interleaved into the ACT-bound attention
stream of batch b so the PE fills its exp-wait gaps.

All matmul operands are bf16 (f32 PSUM accumulate): this halves HBM/DMA
traffic, removes the fp32r 4x penalty on sub-256 free dims, and doubles DVE
throughput on elementwise tiles. V is produced token-major directly from the
QKV matmul (stationary = x tile, moving = W_v), removing all PE transposes.
The causal mask is applied by multiplying the exp'd diagonal block with a
precomputed 0/1 lower-triangular bf16 mask; the attention v-bias is folded
into the projection bias on the host (b_proj + b_v @ W_proj). DMAs are merged
into few large strided transfers split across the SP and Pool queues so no
single sequencer serializes the issue path.
"""

import math
from contextlib import ExitStack

import numpy as np

NCORES = 8
B, T, C = 4, 2048, 1024
H = 16
D = C // H  # 64
HPC = H // NCORES  # heads per core = 2
BT = B * T  # 8192
ROWS_PER_CORE = BT // NCORES  # 1024
NKT = T // 128  # 16 k-tiles per batch
QROWS = T // NCORES  # 256

_compiled = None


def _build(no_collective=False):
    import concourse.tile as tile
    from concourse import bacc, mybir

    f32 = mybir.dt.float32
    bf16 = mybir.dt.bfloat16

    nc = bacc.Bacc()

    # ---- DRAM I/O (per-core views; same kernel on all 8 cores) ----
    xt_d = nc.dram_tensor("xt", [C, BT], bf16, kind="ExternalInput")
    wqkv_d = nc.dram_tensor("wqkv", [C, 3 * 128], bf16, kind="ExternalInput")
    bqk_d = nc.dram_tensor("bqk", [128, 2], f32, kind="ExternalInput")
    wp_d = nc.dram_tensor("wp", [C, C], bf16, kind="ExternalInput")
    bp_d = nc.dram_tensor("bp", [1, C], f32, kind="ExternalInput")
    out_d = nc.dram_tensor("out", [ROWS_PER_CORE, C], f32, kind="ExternalOutput")

    # internal DRAM for the four collectives (one per batch; shard = 256 rows)
    y_loc = [nc.dram_tensor(f"y_loc{q}", [NCORES, 128, QROWS], bf16) for q in range(B)]
    y_all = [nc.dram_tensor(f"y_all{q}", [NCORES, 128, QROWS], bf16) for q in range(B)]

    xt_r = xt_d[:, :].rearrange("(j p) t -> p j t", p=128)  # [128, 8, BT]
    wqkv_r = wqkv_d[:, :].rearrange("(j p) f -> p j f", p=128)  # [128, 8, 384]
    wp_r = wp_d[:, :].rearrange("(j p) f -> p j f", p=128)  # [128, 8, 1024]

    with tile.TileContext(nc) as tc, ExitStack() as ctx:
        qkv_pool = ctx.enter_context(tc.tile_pool(name="qkv_pool", bufs=2))
        wpool = ctx.enter_context(tc.tile_pool(name="wpool", bufs=1))
        xt_pool = ctx.enter_context(tc.tile_pool(name="xt_pool", bufs=3))
        pt_pool = ctx.enter_context(tc.tile_pool(name="pt", bufs=12))
        r_pool = ctx.enter_context(tc.tile_pool(name="rp", bufs=8))
        yt_pool = ctx.enter_context(tc.tile_pool(name="yt", bufs=6))
        ytr_pool = ctx.enter_context(tc.tile_pool(name="ytr", bufs=6))
        out_pool = ctx.enter_context(tc.tile_pool(name="op", bufs=4))
        ya_pool = ctx.enter_context(tc.tile_pool(name="ya", bufs=4))
        ps_big = ctx.enter_context(tc.tile_pool(name="ps_big", bufs=3, space="PSUM"))
        ps_ya = ctx.enter_context(tc.tile_pool(name="ps_ya", bufs=1, space="PSUM"))
        ps_yb = ctx.enter_context(tc.tile_pool(name="ps_yb", bufs=1, space="PSUM"))

        # ---- weights + constants ----
        wq_sb = wpool.tile([128, 8, 3 * 128], bf16)
        bqk_sb = wpool.tile([128, 2], f32)

        def emit_wq_load():
            # q+k weights first on SP (ahead of xt) so the first matmul can
            # start as early as possible; the v part follows
            nc.sync.dma_start(out=wq_sb[:, :, 0:256], in_=wqkv_r[:, :, 0:256])
            nc.sync.dma_start(out=bqk_sb, in_=bqk_d[:, :])

        def emit_wqv_load():
            nc.sync.dma_start(out=wq_sb[:, :, 256:384], in_=wqkv_r[:, :, 256:384])

        # 0/1 lower-triangular (incl diagonal) bf16 mask in [k=partition,
        # q=free] orientation: keep q >= k.
        trimask = wpool.tile([128, 2, 128], bf16, tag="trimask")
        wp_sb = wpool.tile([128, 8, C], bf16)
        bp_row = wpool.tile([128, C], f32, tag="bp_row")
        bias_bc = wpool.tile([128, C], f32, tag="bias_bc")

        def emit_consts():
            nc.gpsimd.memset(trimask[:, :, :], 1.0)
            for s in range(2):
                nc.gpsimd.affine_select(
                    out=trimask[:, s, :],
                    in_=trimask[:, s, :],
                    compare_op=mybir.AluOpType.is_ge,
                    fill=0.0,
                    base=0,
                    pattern=[[1, 128]],
                    channel_multiplier=-1,
                )

        def emit_wp_load():
            nc.sync.dma_start(out=wp_sb[:, :, :], in_=wp_r[:, :, :])
            nc.sync.dma_start(out=bp_row[0:1, :], in_=bp_d[:, :])
            nc.gpsimd.partition_broadcast(bias_bc[:, :], bp_row[0:1, :])

        def phase1(b):
            """QKV projection for batch b (generator; 12 yields). qT/kT are
            feature-major [128=(s,d), T]; V is token-major [128=kpos, kt,
            130] with all-ones columns at 0 (slot 0) and 65 (slot 1) so the
            AV matmul's psum row 0 is the softmax denominator."""
            qT = qkv_pool.tile([128, T], bf16, tag="qT", name=f"qT{b}")
            kT = qkv_pool.tile([128, T], bf16, tag="kT", name=f"kT{b}")
            V = qkv_pool.tile([128, NKT, 130], bf16, tag="V", name=f"V{b}")
            nc.gpsimd.memset(V[:, :, 64], 1.0)
            nc.gpsimd.memset(V[:, :, 129], 1.0)
            result[b] = (qT, kT, V)

            xt_tiles = {}

            def load_xt(tt):
                tok0 = b * T + tt * 512
                xt_t = xt_pool.tile([128, 8, 512], bf16, tag="xt", name=f"xt{b}_{tt}")
                if b == 0 and tt == 0:
                    # split the cold-start load so the first j-chunks land early
                    nc.sync.dma_start(out=xt_t[:, 0:2, :], in_=xt_r[:, 0:2, tok0 : tok0 + 512])
                    nc.sync.dma_start(out=xt_t[:, 2:8, :], in_=xt_r[:, 2:8, tok0 : tok0 + 512])
                else:
                    nc.sync.dma_start(out=xt_t[:, :, :], in_=xt_r[:, :, tok0 : tok0 + 512])
                xt_tiles[tt] = xt_t

            if b == 0:
                emit_wq_load()
            load_xt(0)
            if b == 0:
                emit_wqv_load()
            for tt in range(4):  # 512-token tiles
                if tt + 1 < 4:
                    load_xt(tt + 1)  # prefetch one tile ahead
                xt_t = xt_tiles.pop(tt)
                for m in range(2):  # q, k feature chunks (feature-major)
                    ps = ps_big.tile([128, 2, 512], f32, tag="big")
                    for j in range(8):
                        nc.tensor.matmul(
                            ps[:, 0, :],
                            wq_sb[:, j, m * 128 : (m + 1) * 128],
                            xt_t[:, j, :],
                            start=(j == 0),
                            stop=(j == 7),
                        )
                        if j == 3:
                            yield
                    dst = qT if m == 0 else kT
                    nc.vector.tensor_scalar_add(
                        dst[:, tt * 512 : (tt + 1) * 512], ps[:, 0, :], bqk_sb[:, m : m + 1]
                    )
                    yield
                # v chunk, token-major: stationary = x tile, moving = W_v
                ps = ps_big.tile([128, 2, 512], f32, tag="big")
                for i in range(4):
                    for j in range(8):
                        nc.tensor.matmul(
                            ps[:, 0, i * 128 : (i + 1) * 128],
                            xt_t[:, j, i * 128 : (i + 1) * 128],
                            wq_sb[:, j, 256:384],
                            start=(j == 0),
                            stop=(j == 7),
                        )
                    kt_idx = tt * 4 + i
                    nc.vector.tensor_copy(
                        V[:, kt_idx, 0:64], ps[:, 0, i * 128 : i * 128 + 64]
                    )
                    nc.vector.tensor_copy(
                        V[:, kt_idx, 65:129], ps[:, 0, i * 128 + 64 : i * 128 + 128]
                    )
                    if i % 2 == 1:
                        yield

        def phase2(b):
            """Causal attention for batch b, both head-slots packed into each
            512-query supertile strip: one exp instruction covers both
            slots' scores via a 3D access pattern."""
            qT, kT, V = result[b]
            for qsup in range(4):  # 512-wide query supertiles
                yield True  # filler hint before the supertile's first strip
                q0 = qsup * 512
                ps_y = [
                    ps_ya.tile([128, 512], f32, tag="ya", name="ps_ya"),
                    ps_yb.tile([128, 512], f32, tag="yb", name="ps_yb"),
                ]
                nkt = 4 * (qsup + 1)
                pending = []

                def flush_one():
                    pt, kt, off = pending.pop(0)
                    for s in range(2):
                        nc.tensor.matmul(
                            ps_y[s][0:65, off:512],
                            V[:, kt, s * 65 : s * 65 + 65],
                            pt[:, s, off:512],
                            start=(kt == 0),
                            stop=(kt == nkt - 1),
                        )

                for kt in range(nkt):
                    off = max(0, kt * 128 - q0)
                    ps_s = ps_big.tile([128, 2, 512], f32, tag="big")
                    for s in range(2):
                        p0 = s * D
                        nc.tensor.matmul(
                            ps_s[:, s, off:512],
                            kT[p0 : p0 + D, kt * 128 : (kt + 1) * 128],
                            qT[p0 : p0 + D, q0 + off : q0 + 512],
                            start=True,
                            stop=True,
                        )
                    pt = pt_pool.tile([128, 2, 512], bf16)
                    nc.scalar.activation(
                        pt[:, :, off:512],
                        ps_s[:, :, off:512],
                        mybir.ActivationFunctionType.Exp,
                        scale=1.0 / math.sqrt(D),
                    )
                    if kt * 128 >= q0:  # diagonal strip -> causal mask,
                        # both slots in one op via the duplicated mask
                        nc.vector.tensor_mul(
                            pt[:, :, off : off + 128],
                            pt[:, :, off : off + 128],
                            trimask[:, :, :],
                        )
                    pending.append((pt, kt, off))
                    if len(pending) > 2:  # 2-strip AV skew
                        flush_one()
                    yield True
                while pending:
                    flush_one()

                # free the psum accumulators quickly with copies (row 64 is
                # the denominator), then normalize from SBUF off the
                # slot-critical path
                for s in range(2):
                    p0 = s * D
                    yt_raw = ytr_pool.tile([128, 512], bf16)
                    nc.vector.tensor_copy(yt_raw[0:65, :], ps_y[s][0:65, :])
                    r_t = r_pool.tile([128, 512], bf16, tag="r")
                    with nc.allow_low_precision(reason="softmax denom recip bf16"):
                        nc.vector.reciprocal(r_t[0:1, :], yt_raw[64:65, :])
                    rb_t = r_pool.tile([128, 512], bf16, tag="rb")
                    nc.gpsimd.partition_broadcast(rb_t[0:64, :], r_t[0:1, :])
                    yt_sb = yt_pool.tile([128, 512], bf16)
                    nc.vector.tensor_mul(yt_sb[0:64, :], yt_raw[0:64, :], rb_t[0:64, :])
                    nc.sync.dma_start(
                        out=y_loc[b][qsup * 2 : (qsup + 1) * 2, p0 : p0 + D, :].rearrange(
                            "s p c -> p s c"
                        ),
                        in_=yt_sb[0:64, :],
                    )

        def emit_a2a(q):
            if no_collective:
                return
            nc.gpsimd.collective_compute(
                "AllToAll",
                mybir.AluOpType.bypass,
                replica_groups=[list(range(NCORES))],
                ins=[y_loc[q][:, :, :]],
                outs=[y_all[q][:, :, :]],
            )

        ya_tiles = {}
        warm_src = [None]

        def emit_ya_load(q):
            """Issue the y redistribution load right after batch q's A2A so
            proj never head-of-line-blocks the PE queue on it."""
            y_src = y_loc[q] if no_collective else y_all[q]
            ya = ya_pool.tile([128, 8, QROWS], bf16, tag="ya", name=f"ya{q}")
            nc.sync.dma_start(out=ya, in_=y_src[:, :, :].rearrange("i p t -> p i t"))
            ya_tiles[q] = ya

        def proj(q):
            """Output projection for this core's 256 rows of batch q.
            Generator: yields after each of 2 row-tiles."""
            ya = ya_tiles.pop(q)
            warm_src[0] = ya
            for rt in range(2):
                ps_o = ps_big.tile([128, 2, 512], f32, tag="big")
                out_sb = out_pool.tile([128, 1024], f32)
                row = q * 256 + rt * 128
                # seg-outer so each 512-half drains while the next computes
                for g, (lo, hi) in enumerate(((0, 512), (512, 1024))):
                    for i in range(8):  # feature chunks (source cores)
                        nc.tensor.matmul(
                            ps_o[:, g, :],
                            ya[:, i, rt * 128 : (rt + 1) * 128],
                            wp_sb[:, i, lo:hi],
                            start=(i == 0),
                            stop=(i == 7),
                        )
                    nc.vector.tensor_add(
                        out_sb[:, lo:hi], ps_o[:, g, :], bias_bc[:, lo:hi]
                    )
                    nc.sync.dma_start(
                        out=out_d[row : row + 128, lo:hi], in_=out_sb[:, lo:hi]
                    )
                yield

        def run_interleaved(primary, filler, pace=(1, 1, 0)):
            """Drain `primary`, advancing `filler` at hinted insertion
            points, pacing feeds by the cyclic `pace` pattern so filler
            work spreads across the whole attention stream instead of
            front-loading and running dry."""
            i = 0
            for hint in primary:
                if filler is not None and hint and pace[i % len(pace)]:
                    try:
                        next(filler)
                    except StopIteration:
                        filler = None
                i += 1
            return filler

        def drain(gen):
            if gen is not None:
                for _ in gen:
                    pass

        def chain(*gens):
            for g in gens:
                if g is not None:
                    yield from g

        def take(gen, n):
            for _ in range(n):
                try:
                    next(gen)
                except StopIteration:
                    return
                yield

        result = {}
        # startup: only the first half (tt0, tt1) of batch 0's qkv stands
        # alone. Every batch's attention then gets as filler: the second
        # half of its OWN qkv (tt2, tt3 — the batch-0 software-pipeline
        # pattern applied to all batches), the first half of the NEXT
        # batch's qkv, and the projection of batch b-1. This spreads PE
        # filler evenly so the ACT-bound attention never starves the PE —
        # including batch 3, which previously ran dry.
        p1 = {0: phase1(0)}
        next(p1[0])
        emit_consts()
        for _ in range(11):
            next(p1[0])
        for b in range(B):
            parts = [p1[b]]  # rest of this batch's qkv (tt2, tt3)
            if b < B - 1:
                p1[b + 1] = phase1(b + 1)
                parts.append(take(p1[b + 1], 12))  # next batch's tt0, tt1
            if 1 <= b <= 2:
                parts.append(proj(b - 1))
            filler = chain(*parts)
            pace = (1, 1, 0) if b == 0 else ((1, 0) if b < 3 else (1, 0, 0))
            filler = run_interleaved(phase2(b), filler, pace)
            drain(filler)
            if b == 0:
                emit_wp_load()  # off the critical startup path
            emit_a2a(b)
            emit_ya_load(b)
        drain(proj(2))
        ya2 = warm_src[0]
        for w in range(1):
            warm = ps_big.tile([128, 1024], f32, tag="big")
            for i in range(8):
                nc.tensor.matmul(
                    warm[:, 0:512],
                    ya2[:, i, 0:128],
                    wp_sb[:, i, 0:512],
                    start=(i == 0),
                    stop=(i == 7),
                )
                nc.tensor.matmul(
                    warm[:, 512:1024],
                    ya2[:, i, 0:128],
                    wp_sb[:, i, 512:1024],
                    start=(i == 0),
                    stop=(i == 7),
                )
        drain(proj(3))

    nc.compile()
    return nc


def _get_compiled():
    global _compiled
    if _compiled is None:
        _compiled = _build()
    return _compiled


def _bf16(a):
    import ml_dtypes

    return np.ascontiguousarray(a).astype(ml_dtypes.bfloat16)


def _make_in_maps(x, W_attn, b_attn, W_proj, b_proj):
    xt = _bf16(x.reshape(BT, C).T)  # [C, BT] bf16, shared across cores
    wp = _bf16(W_proj)
    # fold the attention v-bias into the projection bias:
    # y_final = (attn_out + b_v) @ W_proj + b_proj
    bp = np.ascontiguousarray(
        (b_proj + b_attn[2 * C : 3 * C] @ W_proj).reshape(1, C).astype(np.float32)
    )
    in_maps = []
    for c in range(NCORES):
        heads = [HPC * c + s for s in range(HPC)]
        cols = []
        for m in range(3):  # q, k, v blocks of W_attn
            for h in heads:
                cols.extend(range(m * C + h * D, m * C + (h + 1) * D))
        cols = np.asarray(cols)
        bqk = np.ascontiguousarray(
            b_attn[cols].reshape(3, 128).T[:, 0:2].astype(np.float32)
        )
        in_maps.append(
            {
                "xt": xt,
                "wqkv": _bf16(W_attn[:, cols]),
                "bqk": bqk,
                "wp": wp,
                "bp": bp,
            }
        )
    return in_maps


def kernel(x, W_attn, b_attn, W_proj, b_proj):
    from concourse.bass_utils import run_bass_kernel_spmd

    x = np.asarray(x, dtype=np.float32)
    W_attn = np.asarray(W_attn, dtype=np.float32)
    b_attn = np.asarray(b_attn, dtype=np.float32)
    W_proj = np.asarray(W_proj, dtype=np.float32)
    b_proj = np.asarray(b_proj, dtype=np.float32)

    nc = _get_compiled()
    in_maps = _make_in_maps(x, W_attn, b_attn, W_proj, b_proj)
    res = run_bass_kernel_spmd(nc, in_maps, core_ids=list(range(NCORES)))

    # core c's output: for each batch q, rows [256c, 256c+256) of that batch
    out = np.empty((BT, C), dtype=np.float32)
    for c in range(NCORES):
        o = res.results[c]["out"]
        for q in range(B):
            out[2048 * q + 256 * c : 2048 * q + 256 * (c + 1)] = o[256 * q : 256 * (q + 1)]
    return out.reshape(B, T, C)


# revision 68
# speedup vs baseline: 1.0073x; 1.0067x over previous
"""Causal self-attention (B=4, T=2048, C=1024, H=16) on 8 trn2 NeuronCores.

Sharding: 2 heads per core for QKV+attention (tensor-parallel over heads);
per-batch AllToAlls redistribute per-head attention outputs into per-core row
slices for a row-parallel output projection. QKV projection of batch b+1 and
the projection of batch b-1 are interleaved into the ACT-bound attention
stream of batch b so the PE fills its exp-wait gaps.

All matmul operands are bf16 (f32 PSUM accumulate): this halves HBM/DMA
traffic, removes the fp32r 4x penalty on sub-256 free dims, and doubles DVE
throughput on elementwise tiles. V is produced token-major directly from the
QKV matmul (stationary = x tile, moving = W_v), removing all PE transposes.
The causal mask is applied by multiplying the exp'd diagonal block with a
precomputed 0/1 lower-triangular bf16 mask; the attention v-bias is folded
into the projection bias on the host (b_proj + b_v @ W_proj). DMAs are merged
into few large strided transfers split across the SP and Pool queues so no
single sequencer serializes the issue path.
"""

import math
from contextlib import ExitStack

import numpy as np

NCORES = 8
B, T, C = 4, 2048, 1024
H = 16
D = C // H  # 64
HPC = H // NCORES  # heads per core = 2
BT = B * T  # 8192
ROWS_PER_CORE = BT // NCORES  # 1024
NKT = T // 128  # 16 k-tiles per batch
QROWS = T // NCORES  # 256

_compiled = None


def _build(no_collective=False):
    import concourse.tile as tile
    from concourse import bacc, mybir

    f32 = mybir.dt.float32
    bf16 = mybir.dt.bfloat16

    nc = bacc.Bacc()

    # ---- DRAM I/O (per-core views; same kernel on all 8 cores) ----
    xt_d = nc.dram_tensor("xt", [C, BT], bf16, kind="ExternalInput")
    wqkv_d = nc.dram_tensor("wqkv", [C, 3 * 128], bf16, kind="ExternalInput")
    bqk_d = nc.dram_tensor("bqk", [128, 2], f32, kind="ExternalInput")
    wp_d = nc.dram_tensor("wp", [C, C], bf16, kind="ExternalInput")
    bp_d = nc.dram_tensor("bp", [1, C], f32, kind="ExternalInput")
    out_d = nc.dram_tensor("out", [ROWS_PER_CORE, C], f32, kind="ExternalOutput")

    # internal DRAM for the four collectives (one per batch; shard = 256 rows)
    y_loc = [nc.dram_tensor(f"y_loc{q}", [NCORES, 128, QROWS], bf16) for q in range(B)]
    y_all = [nc.dram_tensor(f"y_all{q}", [NCORES, 128, QROWS], bf16) for q in range(B)]

    xt_r = xt_d[:, :].rearrange("(j p) t -> p j t", p=128)  # [128, 8, BT]
    wqkv_r = wqkv_d[:, :].rearrange("(j p) f -> p j f", p=128)  # [128, 8, 384]
    wp_r = wp_d[:, :].rearrange("(j p) f -> p j f", p=128)  # [128, 8, 1024]

    with tile.TileContext(nc) as tc, ExitStack() as ctx:
        qkv_pool = ctx.enter_context(tc.tile_pool(name="qkv_pool", bufs=2))
        wpool = ctx.enter_context(tc.tile_pool(name="wpool", bufs=1))
        xt_pool = ctx.enter_context(tc.tile_pool(name="xt_pool", bufs=3))
        pt_pool = ctx.enter_context(tc.tile_pool(name="pt", bufs=12))
        r_pool = ctx.enter_context(tc.tile_pool(name="rp", bufs=8))
        yt_pool = ctx.enter_context(tc.tile_pool(name="yt", bufs=6))
        ytr_pool = ctx.enter_context(tc.tile_pool(name="ytr", bufs=6))
        out_pool = ctx.enter_context(tc.tile_pool(name="op", bufs=4))
        ya_pool = ctx.enter_context(tc.tile_pool(name="ya", bufs=4))
        ps_big = ctx.enter_context(tc.tile_pool(name="ps_big", bufs=3, space="PSUM"))
        ps_ya = ctx.enter_context(tc.tile_pool(name="ps_ya", bufs=1, space="PSUM"))
        ps_yb = ctx.enter_context(tc.tile_pool(name="ps_yb", bufs=1, space="PSUM"))

        # ---- weights + constants ----
        wq_sb = wpool.tile([128, 8, 3 * 128], bf16)
        bqk_sb = wpool.tile([128, 2], f32)

        def emit_wq_load():
            # q+k weights first on SP (ahead of xt) so the first matmul can
            # start as early as possible; the v part follows
            nc.sync.dma_start(out=wq_sb[:, :, 0:256], in_=wqkv_r[:, :, 0:256])
            nc.sync.dma_start(out=bqk_sb, in_=bqk_d[:, :])

        def emit_wqv_load():
            nc.sync.dma_start(out=wq_sb[:, :, 256:384], in_=wqkv_r[:, :, 256:384])

        # 0/1 lower-triangular (incl diagonal) bf16 mask in [k=partition,
        # q=free] orientation: keep q >= k.
        trimask = wpool.tile([128, 2, 128], bf16, tag="trimask")
        wp_sb = wpool.tile([128, 8, C], bf16)
        bp_row = wpool.tile([128, C], f32, tag="bp_row")
        bias_bc = wpool.tile([128, C], f32, tag="bias_bc")

        def emit_consts():
            nc.gpsimd.memset(trimask[:, :, :], 1.0)
            for s in range(2):
                nc.gpsimd.affine_select(
                    out=trimask[:, s, :],
                    in_=trimask[:, s, :],
                    compare_op=mybir.AluOpType.is_ge,
                    fill=0.0,
                    base=0,
                    pattern=[[1, 128]],
                    channel_multiplier=-1,
                )

        def emit_wp_load():
            nc.sync.dma_start(out=wp_sb[:, :, :], in_=wp_r[:, :, :])
            nc.sync.dma_start(out=bp_row[0:1, :], in_=bp_d[:, :])
            nc.gpsimd.partition_broadcast(bias_bc[:, :], bp_row[0:1, :])

        def phase1(b):
            """QKV projection for batch b (generator; 12 yields). qT/kT are
            feature-major [128=(s,d), T]; V is token-major [128=kpos, kt,
            130] with all-ones columns at 0 (slot 0) and 65 (slot 1) so the
            AV matmul's psum row 0 is the softmax denominator."""
            qT = qkv_pool.tile([128, T], bf16, tag="qT", name=f"qT{b}")
            kT = qkv_pool.tile([128, T], bf16, tag="kT", name=f"kT{b}")
            V = qkv_pool.tile([128, NKT, 130], bf16, tag="V", name=f"V{b}")
            nc.gpsimd.memset(V[:, :, 64], 1.0)
            nc.gpsimd.memset(V[:, :, 129], 1.0)
            result[b] = (qT, kT, V)

            xt_tiles = {}

            def load_xt(tt):
                tok0 = b * T + tt * 512
                xt_t = xt_pool.tile([128, 8, 512], bf16, tag="xt", name=f"xt{b}_{tt}")
                if b == 0 and tt == 0:
                    # split the cold-start load so the first j-chunks land early
                    nc.sync.dma_start(out=xt_t[:, 0:2, :], in_=xt_r[:, 0:2, tok0 : tok0 + 512])
                    nc.sync.dma_start(out=xt_t[:, 2:8, :], in_=xt_r[:, 2:8, tok0 : tok0 + 512])
                else:
                    nc.sync.dma_start(out=xt_t[:, :, :], in_=xt_r[:, :, tok0 : tok0 + 512])
                xt_tiles[tt] = xt_t

            if b == 0:
                emit_wq_load()
            load_xt(0)
            if b == 0:
                emit_wqv_load()
            for tt in range(4):  # 512-token tiles
                if tt + 1 < 4:
                    load_xt(tt + 1)  # prefetch one tile ahead
                xt_t = xt_tiles.pop(tt)
                for m in range(2):  # q, k feature chunks (feature-major)
                    ps = ps_big.tile([128, 2, 512], f32, tag="big")
                    for j in range(8):
                        nc.tensor.matmul(
                            ps[:, 0, :],
                            wq_sb[:, j, m * 128 : (m + 1) * 128],
                            xt_t[:, j, :],
                            start=(j == 0),
                            stop=(j == 7),
                        )
                        if j == 3:
                            yield
                    dst = qT if m == 0 else kT
                    nc.vector.tensor_scalar_add(
                        dst[:, tt * 512 : (tt + 1) * 512], ps[:, 0, :], bqk_sb[:, m : m + 1]
                    )
                    yield
                # v chunk, token-major: stationary = x tile, moving = W_v
                ps = ps_big.tile([128, 2, 512], f32, tag="big")
                for i in range(4):
                    for j in range(8):
                        nc.tensor.matmul(
                            ps[:, 0, i * 128 : (i + 1) * 128],
                            xt_t[:, j, i * 128 : (i + 1) * 128],
                            wq_sb[:, j, 256:384],
                            start=(j == 0),
                            stop=(j == 7),
                        )
                    kt_idx = tt * 4 + i
                    nc.vector.tensor_copy(
                        V[:, kt_idx, 0:64], ps[:, 0, i * 128 : i * 128 + 64]
                    )
                    nc.vector.tensor_copy(
                        V[:, kt_idx, 65:129], ps[:, 0, i * 128 + 64 : i * 128 + 128]
                    )
                    if i % 2 == 1:
                        yield

        def phase2(b):
            """Causal attention for batch b, both head-slots packed into each
            512-query supertile strip: one exp instruction covers both
            slots' scores via a 3D access pattern."""
            qT, kT, V = result[b]
            for qsup in range(4):  # 512-wide query supertiles
                yield True  # filler hint before the supertile's first strip
                q0 = qsup * 512
                ps_y = [
                    ps_ya.tile([128, 512], f32, tag="ya", name="ps_ya"),
                    ps_yb.tile([128, 512], f32, tag="yb", name="ps_yb"),
                ]
                nkt = 4 * (qsup + 1)
                pending = []

                def flush_one():
                    pt, kt, off = pending.pop(0)
                    for s in range(2):
                        nc.tensor.matmul(
                            ps_y[s][0:65, off:512],
                            V[:, kt, s * 65 : s * 65 + 65],
                            pt[:, s, off:512],
                            start=(kt == 0),
                            stop=(kt == nkt - 1),
                        )

                for kt in range(nkt):
                    off = max(0, kt * 128 - q0)
                    ps_s = ps_big.tile([128, 2, 512], f32, tag="big")
                    for s in range(2):
                        p0 = s * D
                        nc.tensor.matmul(
                            ps_s[:, s, off:512],
                            kT[p0 : p0 + D, kt * 128 : (kt + 1) * 128],
                            qT[p0 : p0 + D, q0 + off : q0 + 512],
                            start=True,
                            stop=True,
                        )
                    pt = pt_pool.tile([128, 2, 512], bf16)
                    nc.scalar.activation(
                        pt[:, :, off:512],
                        ps_s[:, :, off:512],
                        mybir.ActivationFunctionType.Exp,
                        scale=1.0 / math.sqrt(D),
                    )
                    if kt * 128 >= q0:  # diagonal strip -> causal mask,
                        # both slots in one op via the duplicated mask
                        nc.vector.tensor_mul(
                            pt[:, :, off : off + 128],
                            pt[:, :, off : off + 128],
                            trimask[:, :, :],
                        )
                    pending.append((pt, kt, off))
                    if len(pending) > 2:  # 2-strip AV skew
                        flush_one()
                    yield True
                while pending:
                    flush_one()

                # free the psum accumulators quickly with copies (row 64 is
                # the denominator), then normalize from SBUF off the
                # slot-critical path
                for s in range(2):
                    p0 = s * D
                    yt_raw = ytr_pool.tile([128, 512], bf16)
                    nc.vector.tensor_copy(yt_raw[0:65, :], ps_y[s][0:65, :])
                    r_t = r_pool.tile([128, 512], bf16, tag="r")
                    with nc.allow_low_precision(reason="softmax denom recip bf16"):
                        nc.vector.reciprocal(r_t[0:1, :], yt_raw[64:65, :])
                    rb_t = r_pool.tile([128, 512], bf16, tag="rb")
                    nc.gpsimd.partition_broadcast(rb_t[0:64, :], r_t[0:1, :])
                    yt_sb = yt_pool.tile([128, 512], bf16)
                    nc.vector.tensor_mul(yt_sb[0:64, :], yt_raw[0:64, :], rb_t[0:64, :])
                    nc.sync.dma_start(
                        out=y_loc[b][qsup * 2 : (qsup + 1) * 2, p0 : p0 + D, :].rearrange(
                            "s p c -> p s c"
                        ),
                        in_=yt_sb[0:64, :],
                    )

        def emit_a2a(q):
            if no_collective:
                return
            nc.gpsimd.collective_compute(
                "AllToAll",
                mybir.AluOpType.bypass,
                replica_groups=[list(range(NCORES))],
                ins=[y_loc[q][:, :, :]],
                outs=[y_all[q][:, :, :]],
            )

        ya_tiles = {}
        warm_src = [None]

        def emit_ya_load(q):
            """Issue the y redistribution load right after batch q's A2A so
            proj never head-of-line-blocks the PE queue on it."""
            y_src = y_loc[q] if no_collective else y_all[q]
            ya = ya_pool.tile([128, 8, QROWS], bf16, tag="ya", name=f"ya{q}")
            nc.sync.dma_start(out=ya, in_=y_src[:, :, :].rearrange("i p t -> p i t"))
            ya_tiles[q] = ya

        def proj(q):
            """Output projection for this core's 256 rows of batch q.
            Generator: yields after each of 2 row-tiles."""
            ya = ya_tiles.pop(q)
            warm_src[0] = ya
            for rt in range(2):
                ps_o = ps_big.tile([128, 2, 512], f32, tag="big")
                out_sb = out_pool.tile([128, 1024], f32)
                row = q * 256 + rt * 128
                # seg-outer so each 512-half drains while the next computes
                for g, (lo, hi) in enumerate(((0, 512), (512, 1024))):
                    for i in range(8):  # feature chunks (source cores)
                        nc.tensor.matmul(
                            ps_o[:, g, :],
                            ya[:, i, rt * 128 : (rt + 1) * 128],
                            wp_sb[:, i, lo:hi],
                            start=(i == 0),
                            stop=(i == 7),
                        )
                    nc.vector.tensor_add(
                        out_sb[:, lo:hi], ps_o[:, g, :], bias_bc[:, lo:hi]
                    )
                    nc.sync.dma_start(
                        out=out_d[row : row + 128, lo:hi], in_=out_sb[:, lo:hi]
                    )
                yield

        def run_interleaved(primary, filler, pace=(1, 1, 0)):
            """Drain `primary`, advancing `filler` at hinted insertion
            points, pacing feeds by the cyclic `pace` pattern so filler
            work spreads across the whole attention stream instead of
            front-loading and running dry."""
            i = 0
            for hint in primary:
                if filler is not None and hint and pace[i % len(pace)]:
                    try:
                        next(filler)
                    except StopIteration:
                        filler = None
                i += 1
            return filler

        def drain(gen):
            if gen is not None:
                for _ in gen:
                    pass

        def chain(*gens):
            for g in gens:
                if g is not None:
                    yield from g

        def take(gen, n):
            for _ in range(n):
                try:
                    next(gen)
                except StopIteration:
                    return
                yield

        result = {}
        # startup: only the first half (tt0, tt1) of batch 0's qkv stands
        # alone. Every batch's attention then gets as filler: the second
        # half of its OWN qkv (tt2, tt3 — the batch-0 software-pipeline
        # pattern applied to all batches), the first half of the NEXT
        # batch's qkv, and the projection of batch b-1. This spreads PE
        # filler evenly so the ACT-bound attention never starves the PE —
        # including batch 3, which previously ran dry.
        p1 = {0: phase1(0)}
        next(p1[0])
        emit_consts()
        for _ in range(11):
            next(p1[0])
        for b in range(B):
            parts = [p1[b]]  # rest of this batch's qkv (tt2, tt3)
            if b < B - 1:
                p1[b + 1] = phase1(b + 1)
                parts.append(take(p1[b + 1], 12))  # next batch's tt0, tt1
            if 1 <= b <= 2:
                parts.append(proj(b - 1))
            filler = chain(*parts)
            pace = (1, 0) if b < 3 else (1, 0, 1, 0, 0)
            filler = run_interleaved(phase2(b), filler, pace)
            drain(filler)
            if b == 0:
                emit_wp_load()  # off the critical startup path
            emit_a2a(b)
            emit_ya_load(b)
        drain(proj(2))
        ya2 = warm_src[0]
        for w in range(1):
            warm = ps_big.tile([128, 1024], f32, tag="big")
            for i in range(8):
                nc.tensor.matmul(
                    warm[:, 0:512],
                    ya2[:, i, 0:128],
                    wp_sb[:, i, 0:512],
                    start=(i == 0),
                    stop=(i == 7),
                )
                nc.tensor.matmul(
                    warm[:, 512:1024],
                    ya2[:, i, 0:128],
                    wp_sb[:, i, 512:1024],
                    start=(i == 0),
                    stop=(i == 7),
                )
        drain(proj(3))

    nc.compile()
    return nc


def _get_compiled():
    global _compiled
    if _compiled is None:
        _compiled = _build()
    return _compiled


def _bf16(a):
    import ml_dtypes

    return np.ascontiguousarray(a).astype(ml_dtypes.bfloat16)


def _make_in_maps(x, W_attn, b_attn, W_proj, b_proj):
    xt = _bf16(x.reshape(BT, C).T)  # [C, BT] bf16, shared across cores
    wp = _bf16(W_proj)
    # fold the attention v-bias into the projection bias:
    # y_final = (attn_out + b_v) @ W_proj + b_proj
    bp = np.ascontiguousarray(
        (b_proj + b_attn[2 * C : 3 * C] @ W_proj).reshape(1, C).astype(np.float32)
    )
    in_maps = []
    for c in range(NCORES):
        heads = [HPC * c + s for s in range(HPC)]
        cols = []
        for m in range(3):  # q, k, v blocks of W_attn
            for h in heads:
                cols.extend(range(m * C + h * D, m * C + (h + 1) * D))
        cols = np.asarray(cols)
        bqk = np.ascontiguousarray(
            b_attn[cols].reshape(3, 128).T[:, 0:2].astype(np.float32)
        )
        in_maps.append(
            {
                "xt": xt,
                "wqkv": _bf16(W_attn[:, cols]),
                "bqk": bqk,
                "wp": wp,
                "bp": bp,
            }
        )
    return in_maps


def kernel(x, W_attn, b_attn, W_proj, b_proj):
    from concourse.bass_utils import run_bass_kernel_spmd

    x = np.asarray(x, dtype=np.float32)
    W_attn = np.asarray(W_attn, dtype=np.float32)
    b_attn = np.asarray(b_attn, dtype=np.float32)
    W_proj = np.asarray(W_proj, dtype=np.float32)
    b_proj = np.asarray(b_proj, dtype=np.float32)

    nc = _get_compiled()
    in_maps = _make_in_maps(x, W_attn, b_attn, W_proj, b_proj)
    res = run_bass_kernel_spmd(nc, in_maps, core_ids=list(range(NCORES)))

    # core c's output: for each batch q, rows [256c, 256c+256) of that batch
    out = np.empty((BT, C), dtype=np.float32)
    for c in range(NCORES):
        o = res.results[c]["out"]
        for q in range(B):
            out[2048 * q + 256 * c : 2048 * q + 256 * (c + 1)] = o[256 * q : 256 * (q + 1)]
    return out.reshape(B, T, C)
